# revision 8
# baseline (speedup 1.0000x reference)
"""Bidirectional 2-layer GRU (BS=32, T=2048, D=H=256) on 8 trn2 NeuronCores.

Time-parallel recurrence with warmup (GRU z-gate contraction makes a segment
started from h=0 converge to the true trajectory; W=12 -> ~3e-3 err).

v2 layout: P=32 windows of SEG=64 steps (+W warmup -> R=76 rounds), core
c = (layer c//4, k = c%4); chain g of core k owns 4 windows x 32 batch =
LAN=128 lanes (g0/g1 fwd w 8k..8k+7, g2/g3 bwd same; bwd streams are
host-pre-reversed). NG=4 chains hide the per-step dependency latency while
LAN=128 tiles keep fixed per-instruction overheads small. Engine balance
(per chain-round, steady-state cost-model):
  PE   ~1493ns: K=4-indicator rz-bias seed (fp16, free=512) + 8 Wx_rz +
         4 Wx_n + 12 Wh matmuls, f128/K=128. z-gate rows of Wx/Wh and bz
         are negated on host so sigmoid directly yields z' = 1-z.
  ACT  ~1060ns: sigmoid(ps_rz)->[r,z'], tanh(an)->n.
  DVE  ~1460ns: rn=(ps_hn+bhn)*r and an=(ps_xn+bxn)+rn as 2x
         scalar_tensor_tensor with per-partition bias columns; d=n-h and
         e=d*z' (fp16 sbuf 2x); h' = h+e for g1-g3 (right after e, so the
         round-tail Wh matmuls are not gated by a Pool hop).
  POOL : h' for g0 only (TensorScalarPtr is not legal on Pool; plain tt).
PSUM: per chain rz[128,4,128] + nn[128,4,128] fp32 = 2 banks -> all 8 banks,
bufs=1. Separate rz/nn tiles keep cross-round WAR waits fine-grained (a
fused tile serialized next-round Wx on the slowest previous-round reader).
PE emission: [seed+Wx_rz]*4, [Wx_n]*4 (max WAR slack), [Wh]*4; Wh_rz are
skipped at r=0 (h=0). Boundary streams (fwd w0 / bwd w31) get post-warmup
state zeroed by a mask multiply at round W-1. x is DMA'd per C=4-round
chunk in consumption order; outputs stream out per chunk.
"""

import os
from contextlib import ExitStack

import numpy as np

import concourse.bass as bass
from concourse import mybir
from concourse.alu_op_type import AluOpType
from concourse.tile import TileContext
from concourse.bass_utils import run_bass_kernel_spmd

BS, T_FULL, D = 32, 2048, 256
H, L = 256, 2
NG = 4            # chains per core
LAN = 128         # lanes per chain (4 windows x 32 batch)
NWIN = 4          # windows per chain
PW = 32           # windows per direction
SEG = T_FULL // PW  # 64 steps per window
W = 12            # warmup rounds
R = SEG + W       # 76 rounds
C = 4             # rounds per DMA chunk
NCH = R // C      # 19

F16 = mybir.dt.float16
F32 = mybir.dt.float32
AF = mybir.ActivationFunctionType
AL = AluOpType


def _fix_drain_waits(nc, max_waits=1):
    """Walrus rejects instructions with >1 sync-wait: split extras onto
    single-wait NOPs just before, on the same engine."""
    n_new = 0
    for f in nc.m.functions:
        for bb in f.blocks:
            insts = list(bb.instructions)
            out = []
            changed = False
            for inst in insts:
                si = inst.sync_info
                if si and len(si.on_wait) > max_waits:
                    waits = list(si.on_wait)
                    for k, w in enumerate(waits[:-max_waits]):
                        nd = mybir.InstNoOp(name=f"{inst.name}-w{k}", ins=[], outs=[])
                        nd.engine = inst.engine
                        nd.sync_info = mybir.SyncInfo(on_wait=[w], on_update=[])
                        out.append(nd)
                        nc.register_instruction(nd, overwrite=True)
                        n_new += 1
                    inst.sync_info = mybir.SyncInfo(
                        on_wait=waits[-max_waits:], on_update=list(si.on_update)
                    )
                    changed = True
                out.append(inst)
            if changed:
                lst = bb.instructions
                lst.clear()
                lst.extend(out)
                assert [i.name for i in bb.instructions] == [i.name for i in out]
    return n_new


def _build():
    nc = bass.Bass(name="bidir_gru_v2", trn_type="TRN2")

    # x in consumption order: [ch, p, kx, j, g, lane]
    xt = nc.dram_tensor("xt", [NCH, 128, 2, C, NG, LAN], F16, kind="ExternalInput")
    # weights [p(K-half), kc*6+mt, gate-col] with z-rows negated
    wht = nc.dram_tensor("wht", [128, 12, 128], F16, kind="ExternalInput")
    wxt = nc.dram_tensor("wxt", [128, 12, 128], F16, kind="ExternalInput")
    # K=4 rz-bias seed: b4[reg, p] = bias value of gate-tile reg, col p
    b4d = nc.dram_tensor("b4d", [4, 128], F16, kind="ExternalInput")
    ind4d = nc.dram_tensor("ind4d", [4, 4, LAN], F16, kind="ExternalInput")
    # n-side biases as per-partition scalar columns
    bhn2d = nc.dram_tensor("bhn2d", [128, 2], F32, kind="ExternalInput")
    bxn2d = nc.dram_tensor("bxn2d", [128, 2], F32, kind="ExternalInput")
    hmask = nc.dram_tensor("hmask", [128, 2, NG, LAN], F16, kind="ExternalInput")
    # out[ch, p, j, kc, g, lane]
    out = nc.dram_tensor("out", [NCH, 128, C, 2, NG, LAN], F16, kind="ExternalOutput")

    with TileContext(nc) as tc, ExitStack() as ctx:
        const = ctx.enter_context(tc.tile_pool(name="const", bufs=1))
        xtp = ctx.enter_context(tc.tile_pool(name="xtp", bufs=2))
        outp = ctx.enter_context(tc.tile_pool(name="outp", bufs=2))
        psp = ctx.enter_context(tc.tile_pool(name="psp", bufs=1, space="PSUM"))
        ew = ctx.enter_context(tc.tile_pool(name="ew", bufs=3))

        b4_sb = const.tile([4, 128], F16)
        nc.sync.dma_start(out=b4_sb, in_=b4d[:, :])
        ind4 = const.tile([4, 4, LAN], F16)
        nc.sync.dma_start(out=ind4, in_=ind4d[:, :, :])
        wxt_sb = const.tile([128, 12, 128], F16)
        nc.sync.dma_start(out=wxt_sb, in_=wxt[:, :, :])
        wht_sb = const.tile([128, 12, 128], F16)
        nc.sync.dma_start(out=wht_sb, in_=wht[:, :, :])
        bhn2 = const.tile([128, 2], F32)
        nc.sync.dma_start(out=bhn2, in_=bhn2d[:, :])
        bxn2 = const.tile([128, 2], F32)
        nc.sync.dma_start(out=bxn2, in_=bxn2d[:, :])
        hmask_sb = const.tile([128, 2, NG, LAN], F16)
        nc.sync.dma_start(out=hmask_sb, in_=hmask[:, :, :, :])
        zeros = const.tile([128, 2, NG, LAN], F16)
        nc.vector.memset(zeros, 0.0)

        h_prev = [zeros[:, :, g, :] for g in range(NG)]

        outc = None
        for ch in range(NCH):
            xt_sb = xtp.tile([128, 2, C, NG, LAN], F16, tag="xt")
            nc.sync.dma_start(out=xt_sb, in_=xt[ch])
            outc_prev = outc
            outc = outp.tile([128, C, 2, NG, LAN], F16, tag="outc")
            for j in range(C):
                r = ch * C + j
                rz_all = []
                nn_all = []
                for g in range(NG):
                    # --- PE h-independent phase: rz-bias seed + Wx_rz ---
                    # rz tile: regions 0:4 = [r0, r1, z'0, z'1] (1 PSUM bank)
                    # nn tile: regions 0:2 = hn, 2:4 = xn (1 PSUM bank)
                    rz = psp.tile([128, 4, LAN], F32, tag=f"rz{g}")
                    nn = psp.tile([128, 4, LAN], F32, tag=f"nn{g}")
                    rz_all.append(rz)
                    nn_all.append(nn)
                    # seed rz with biases (z rows negated)
                    nc.tensor.matmul(
                        out=rz.rearrange("p a b -> p (a b)"),
                        lhsT=b4_sb[:, :],
                        rhs=ind4.rearrange("k a b -> k (a b)"),
                        start=True,
                        stop=False,
                    )
                    for mt in range(4):  # Wx_rz
                        for kx in range(2):
                            nc.tensor.matmul(
                                out=rz[:, mt, :],
                                lhsT=wxt_sb[:, kx * 6 + mt, :],
                                rhs=xt_sb[:, kx, j, g, :],
                                start=False,
                                # at r=0 the Wh_rz matmuls are skipped, so
                                # the rz accumulation group ends here
                                stop=(r == 0 and mt == 3 and kx == 1),
                            )
                for g in range(NG):
                    # Wx_n after all rz work: their WAR on the previous
                    # round's rn/an readers gets maximal slack
                    nn = nn_all[g]
                    for mt in range(4, 6):  # Wx_n -> nn regions 2:4
                        for kx in range(2):
                            nc.tensor.matmul(
                                out=nn[:, mt - 2, :],
                                lhsT=wxt_sb[:, kx * 6 + mt, :],
                                rhs=xt_sb[:, kx, j, g, :],
                                start=(kx == 0),
                                stop=False,
                            )
                for g in range(NG):
                    # --- PE recurrent phase: Wh (rz accum, hn fresh) ---
                    # At r=0 the state is exactly zero: skip the rz-side Wh
                    # matmuls (they add nothing); keep Wh_n for the region
                    # start=True init.
                    rz, nn = rz_all[g], nn_all[g]
                    hp = h_prev[g]
                    for mt in range(6):
                        if r == 0 and mt < 4:
                            continue
                        for kc in range(2):
                            nc.tensor.matmul(
                                out=rz[:, mt, :] if mt < 4 else nn[:, mt - 4, :],
                                lhsT=wht_sb[:, kc * 6 + mt, :],
                                rhs=hp[:, kc, :],
                                start=(mt >= 4 and kc == 0),
                                stop=(mt == 3 and kc == 1)
                                if mt < 4
                                else (mt == 5 and kc == 1),
                            )

                sgl = [None] * NG
                rnl = [None] * NG
                anl = [None] * NG
                ntl = [None] * NG
                dl = [None] * NG

                def em_sig(g):
                    # sg = [r0, r1, z'0, z'1]
                    sg = ew.tile([128, 4, LAN], F16, tag=f"sg{g}")
                    nc.scalar.activation(
                        out=sg, in_=rz_all[g], func=AF.Sigmoid
                    )
                    sgl[g] = sg

                def em_rn(g):
                    # rn = (ps_hn + bhn) * r   (2 stt, per-partition bias)
                    rn = ew.tile([128, 2, LAN], F16, tag=f"rn{g}")
                    for kc in range(2):
                        nc.vector.scalar_tensor_tensor(
                            out=rn[:, kc, :],
                            in0=nn_all[g][:, kc, :],
                            scalar=bhn2[:, kc : kc + 1],
                            in1=sgl[g][:, kc, :],
                            op0=AL.add,
                            op1=AL.mult,
                        )
                    rnl[g] = rn

                def em_an(g):
                    # an = (ps_xn + bxn) + rn  (2 stt)
                    an = ew.tile([128, 2, LAN], F16, tag=f"an{g}")
                    for kc in range(2):
                        nc.vector.scalar_tensor_tensor(
                            out=an[:, kc, :],
                            in0=nn_all[g][:, 2 + kc, :],
                            scalar=bxn2[:, kc : kc + 1],
                            in1=rnl[g][:, kc, :],
                            op0=AL.add,
                            op1=AL.add,
                        )
                    anl[g] = an

                def em_tanh(g):
                    nt = ew.tile([128, 2, LAN], F16, tag=f"nt{g}")
                    nc.scalar.activation(out=nt, in_=anl[g], func=AF.Tanh)
                    ntl[g] = nt

                def em_d(g):
                    # d = n - h ; e = d * z'  (DVE, sbuf 2x, back-to-back)
                    d = ew.tile([128, 2, LAN], F16, tag=f"d{g}")
                    nc.vector.tensor_tensor(
                        out=d, in0=ntl[g], in1=h_prev[g], op=AL.subtract
                    )
                    e = ew.tile([128, 2, LAN], F16, tag=f"e{g}")
                    nc.vector.tensor_tensor(
                        out=e, in0=d, in1=sgl[g][:, 2:4, :], op=AL.mult
                    )
                    dl[g] = e

                def em_tail(g):
                    # h' = h + e.  g0 on POOL (its deadline is earliest and
                    # POOL latency fits); g1-g3 on DVE right after their e so
                    # late-round Wh matmuls are not gated by the POOL hop.
                    dst = outc[:, j, :, g, :]
                    eng = nc.vector if g >= 1 else nc.gpsimd
                    eng.tensor_tensor(
                        out=dst, in0=h_prev[g], in1=dl[g], op=AL.add,
                    )
                    h_prev[g] = dst

                # software-pipelined interleave across the 4 chains; DVE
                # order completes ALL rn/an (they gate next round's PE
                # writes) before the d/e tail ops
                em_sig(0)
                em_rn(0)
                em_an(0)
                em_sig(1)
                em_tanh(0)
                em_rn(1)
                em_an(1)
                em_d(0)
                em_sig(2)
                em_tail(0)
                em_tanh(1)
                em_rn(2)
                em_an(2)
                em_d(1)
                em_sig(3)
                em_tail(1)
                em_tanh(2)
                em_rn(3)
                em_an(3)
                em_d(2)
                em_tail(2)
                em_tanh(3)
                em_d(3)
                em_tail(3)

                if r == W - 1:
                    # zero post-warmup state of boundary streams
                    for g in range(NG):
                        hm = outc[:, j, :, g, :]
                        nc.vector.tensor_tensor(
                            out=hm, in0=hm, in1=hmask_sb[:, :, g, :],
                            op=AL.mult,
                        )
                        h_prev[g] = hm
            nc.sync.dma_start(out=out[ch], in_=outc)
            del outc_prev

    _fix_drain_waits(nc)
    return nc


_CACHE = {}


def _get_nc(T=T_FULL):
    assert T == T_FULL, "v2 kernel is specialized to T=2048"
    if T not in _CACHE:
        _CACHE[T] = _build()
    return _CACHE[T]


def prep_in_maps(x, Wx, Wh, bx, bh):
    x = np.asarray(x, np.float32)
    Wx = np.asarray(Wx, np.float32).copy()
    Wh = np.asarray(Wh, np.float32).copy()
    bx = np.asarray(bx, np.float32)
    bh = np.asarray(bh, np.float32)
    T = x.shape[1]
    assert T == T_FULL

    # negate z-gate rows so sigmoid gives z' = 1-z
    Wx[:, 256:512, :] *= -1.0
    Wh[:, 256:512, :] *= -1.0
    brz = (bx + bh)[:, 0:512].copy()
    brz[:, 256:512] *= -1.0

    rr = np.arange(R)
    in_maps = []
    for c in range(8):
        l, k = c // 4, c % 4
        # chains: g0: fwd w 8k+0..3; g1: fwd w 8k+4..7; g2/g3: bwd same
        tidx = np.empty((NG, NWIN, R), np.int64)
        for g in range(NG):
            fwd = g < 2
            for ws in range(NWIN):
                w = 8 * k + (g % 2) * 4 + ws
                if fwd:
                    t = SEG * w - W + rr
                else:
                    t = SEG * (w + 1) - 1 + W - rr
                tidx[g, ws] = np.clip(t, 0, T - 1)
        # gather: [b, g, ws, r, d] -> [ch, p, kx, j, g, (ws, b)]
        xg = x[:, tidx, :].astype(np.float16)  # (32, NG, NWIN, R, 256)
        xg = xg.reshape(BS, NG, NWIN, NCH, C, 2, 128)
        xt_h = np.ascontiguousarray(xg.transpose(3, 6, 5, 4, 1, 2, 0)).reshape(
            NCH, 128, 2, C, NG, LAN
        )

        wht_h = np.ascontiguousarray(
            Wh[l].reshape(6, 128, 2, 128).transpose(3, 2, 0, 1).reshape(128, 12, 128),
            np.float16,
        )
        wxt_h = np.ascontiguousarray(
            Wx[l].reshape(6, 128, 2, 128).transpose(3, 2, 0, 1).reshape(128, 12, 128),
            np.float16,
        )
        b4 = brz[l].reshape(4, 128).astype(np.float16)
        ind4_h = np.zeros((4, 4, LAN), np.float16)
        for kk in range(4):
            ind4_h[kk, kk, :] = 1.0
        bhn2_h = np.ascontiguousarray(bh[l, 512:768].reshape(2, 128).T, np.float32)
        bxn2_h = np.ascontiguousarray(bx[l, 512:768].reshape(2, 128).T, np.float32)

        hm = np.ones((128, 2, NG, LAN), np.float16)
        if k == 0:
            hm[:, :, 0, 0:32] = 0.0  # fwd window 0
        if k == 3:
            hm[:, :, 3, 96:128] = 0.0  # bwd window 31
        in_maps.append(
            {
                "xt": xt_h,
                "wht": wht_h,
                "wxt": wxt_h,
                "b4d": b4,
                "ind4d": ind4_h,
                "bhn2d": bhn2_h,
                "bxn2d": bxn2_h,
                "hmask": hm,
            }
        )
    return in_maps


def assemble_out(per_core_out, T=T_FULL):
    OUT = np.empty((BS, T * L, 2 * H), np.float32)
    for c in range(8):
        l, k = c // 4, c % 4
        o = np.asarray(per_core_out[c], np.float32).reshape(NCH, 128, C, 2, NG, LAN)
        # [ch, p, j, kc, g, lane] -> [r, kc, p, g, ws, b]
        o = o.transpose(0, 2, 3, 1, 4, 5).reshape(R, 2, 128, NG, NWIN, BS)
        o = o.reshape(R, 256, NG, NWIN, BS)
        kept = o[W : W + SEG]  # [seg_j, 256, NG, NWIN, b]
        for g in range(NG):
            fwd = g < 2
            for ws in range(NWIN):
                w = 8 * k + (g % 2) * 4 + ws
                hs = kept[:, :, g, ws, :]  # [seg_j, 256, b]
                if not fwd:
                    hs = hs[::-1]
                ts = np.arange(SEG * w, SEG * (w + 1))
                col0 = 0 if fwd else 256
                OUT[:, 2 * ts + l, col0 : col0 + 256] = hs.transpose(2, 0, 1)
    return OUT


def kernel(x, Wx, Wh, bx, bh):
    T = x.shape[1]
    nc = _get_nc(T)
    in_maps = prep_in_maps(x, Wx, Wh, bx, bh)
    res = run_bass_kernel_spmd(nc, in_maps, core_ids=list(range(8)))
    kernel.last_results = res
    return assemble_out([r["out"] for r in res.results], T)


# revision 9
# speedup vs baseline: 1.0097x; 1.0097x over previous
"""Bidirectional 2-layer GRU (BS=32, T=2048, D=H=256) on 8 trn2 NeuronCores.

Time-parallel recurrence with warmup (GRU z-gate contraction makes a segment
started from h=0 converge to the true trajectory; W=12 -> ~3e-3 err).

v2 layout: P=32 windows of SEG=64 steps (+W warmup -> R=76 rounds), core
c = (layer c//4, k = c%4); chain g of core k owns 4 windows x 32 batch =
LAN=128 lanes (g0/g1 fwd w 8k..8k+7, g2/g3 bwd same; bwd streams are
host-pre-reversed). NG=4 chains hide the per-step dependency latency while
LAN=128 tiles keep fixed per-instruction overheads small. Engine balance
(per chain-round, steady-state cost-model):
  PE   ~1493ns: K=4-indicator rz-bias seed (fp16, free=512) + 8 Wx_rz +
         4 Wx_n + 12 Wh matmuls, f128/K=128. z-gate rows of Wx/Wh and bz
         are negated on host so sigmoid directly yields z' = 1-z.
  ACT  ~1060ns: sigmoid(ps_rz)->[r,z'], tanh(an)->n.
  DVE  ~1460ns: rn=(ps_hn+bhn)*r and an=(ps_xn+bxn)+rn as 2x
         scalar_tensor_tensor with per-partition bias columns; d=n-h and
         e=d*z' (fp16 sbuf 2x); h' = h+e for g1-g3 (right after e, so the
         round-tail Wh matmuls are not gated by a Pool hop).
  POOL : h' for g0 only (TensorScalarPtr is not legal on Pool; plain tt).
PSUM: per chain rz[128,4,128] + nn[128,4,128] fp32 = 2 banks -> all 8 banks,
bufs=1. Separate rz/nn tiles keep cross-round WAR waits fine-grained (a
fused tile serialized next-round Wx on the slowest previous-round reader).
PE emission: [seed+Wx_rz]*4, [Wx_n]*4 (max WAR slack), [Wh]*4; Wh_rz are
skipped at r=0 (h=0). Boundary streams (fwd w0 / bwd w31) get post-warmup
state zeroed by a mask multiply at round W-1. x is DMA'd per C=4-round
chunk in consumption order; outputs stream out per chunk.
"""

import os
from contextlib import ExitStack

import numpy as np

import concourse.bass as bass
from concourse import mybir
from concourse.alu_op_type import AluOpType
from concourse.tile import TileContext
from concourse.bass_utils import run_bass_kernel_spmd

BS, T_FULL, D = 32, 2048, 256
H, L = 256, 2
NG = 4            # chains per core
LAN = 128         # lanes per chain (4 windows x 32 batch)
NWIN = 4          # windows per chain
PW = 32           # windows per direction
SEG = T_FULL // PW  # 64 steps per window
W = 11            # warmup rounds
R = SEG + W       # 76 rounds
C = 5             # rounds per DMA chunk
NCH = R // C      # 19

F16 = mybir.dt.float16
F32 = mybir.dt.float32
AF = mybir.ActivationFunctionType
AL = AluOpType


def _fix_drain_waits(nc, max_waits=1):
    """Walrus rejects instructions with >1 sync-wait: split extras onto
    single-wait NOPs just before, on the same engine."""
    n_new = 0
    for f in nc.m.functions:
        for bb in f.blocks:
            insts = list(bb.instructions)
            out = []
            changed = False
            for inst in insts:
                si = inst.sync_info
                if si and len(si.on_wait) > max_waits:
                    waits = list(si.on_wait)
                    for k, w in enumerate(waits[:-max_waits]):
                        nd = mybir.InstNoOp(name=f"{inst.name}-w{k}", ins=[], outs=[])
                        nd.engine = inst.engine
                        nd.sync_info = mybir.SyncInfo(on_wait=[w], on_update=[])
                        out.append(nd)
                        nc.register_instruction(nd, overwrite=True)
                        n_new += 1
                    inst.sync_info = mybir.SyncInfo(
                        on_wait=waits[-max_waits:], on_update=list(si.on_update)
                    )
                    changed = True
                out.append(inst)
            if changed:
                lst = bb.instructions
                lst.clear()
                lst.extend(out)
                assert [i.name for i in bb.instructions] == [i.name for i in out]
    return n_new


def _build():
    nc = bass.Bass(name="bidir_gru_v2", trn_type="TRN2")

    # x in consumption order: [ch, p, kx, j, g, lane]
    xt = nc.dram_tensor("xt", [NCH, 128, 2, C, NG, LAN], F16, kind="ExternalInput")
    # weights [p(K-half), kc*6+mt, gate-col] with z-rows negated
    wht = nc.dram_tensor("wht", [128, 12, 128], F16, kind="ExternalInput")
    wxt = nc.dram_tensor("wxt", [128, 12, 128], F16, kind="ExternalInput")
    # K=4 rz-bias seed: b4[reg, p] = bias value of gate-tile reg, col p
    b4d = nc.dram_tensor("b4d", [4, 128], F16, kind="ExternalInput")
    ind4d = nc.dram_tensor("ind4d", [4, 4, LAN], F16, kind="ExternalInput")
    # n-side biases as per-partition scalar columns
    bhn2d = nc.dram_tensor("bhn2d", [128, 2], F32, kind="ExternalInput")
    bxn2d = nc.dram_tensor("bxn2d", [128, 2], F32, kind="ExternalInput")
    hmask = nc.dram_tensor("hmask", [128, 2, NG, LAN], F16, kind="ExternalInput")
    # out[ch, p, j, kc, g, lane]
    out = nc.dram_tensor("out", [NCH, 128, C, 2, NG, LAN], F16, kind="ExternalOutput")

    with TileContext(nc) as tc, ExitStack() as ctx:
        const = ctx.enter_context(tc.tile_pool(name="const", bufs=1))
        xtp = ctx.enter_context(tc.tile_pool(name="xtp", bufs=2))
        outp = ctx.enter_context(tc.tile_pool(name="outp", bufs=2))
        psp = ctx.enter_context(tc.tile_pool(name="psp", bufs=1, space="PSUM"))
        ew = ctx.enter_context(tc.tile_pool(name="ew", bufs=3))

        b4_sb = const.tile([4, 128], F16)
        nc.sync.dma_start(out=b4_sb, in_=b4d[:, :])
        ind4 = const.tile([4, 4, LAN], F16)
        nc.sync.dma_start(out=ind4, in_=ind4d[:, :, :])
        wxt_sb = const.tile([128, 12, 128], F16)
        nc.sync.dma_start(out=wxt_sb, in_=wxt[:, :, :])
        wht_sb = const.tile([128, 12, 128], F16)
        nc.sync.dma_start(out=wht_sb, in_=wht[:, :, :])
        bhn2 = const.tile([128, 2], F32)
        nc.sync.dma_start(out=bhn2, in_=bhn2d[:, :])
        bxn2 = const.tile([128, 2], F32)
        nc.sync.dma_start(out=bxn2, in_=bxn2d[:, :])
        hmask_sb = const.tile([128, 2, NG, LAN], F16)
        nc.sync.dma_start(out=hmask_sb, in_=hmask[:, :, :, :])
        zeros = const.tile([128, 2, NG, LAN], F16)
        nc.vector.memset(zeros, 0.0)

        h_prev = [zeros[:, :, g, :] for g in range(NG)]

        outc = None
        for ch in range(NCH):
            xt_sb = xtp.tile([128, 2, C, NG, LAN], F16, tag="xt")
            nc.sync.dma_start(out=xt_sb, in_=xt[ch])
            outc_prev = outc
            outc = outp.tile([128, C, 2, NG, LAN], F16, tag="outc")
            for j in range(C):
                r = ch * C + j
                rz_all = []
                nn_all = []
                for g in range(NG):
                    # --- PE h-independent phase: rz-bias seed + Wx_rz ---
                    # rz tile: regions 0:4 = [r0, r1, z'0, z'1] (1 PSUM bank)
                    # nn tile: regions 0:2 = hn, 2:4 = xn (1 PSUM bank)
                    rz = psp.tile([128, 4, LAN], F32, tag=f"rz{g}")
                    nn = psp.tile([128, 4, LAN], F32, tag=f"nn{g}")
                    rz_all.append(rz)
                    nn_all.append(nn)
                    # seed rz with biases (z rows negated)
                    nc.tensor.matmul(
                        out=rz.rearrange("p a b -> p (a b)"),
                        lhsT=b4_sb[:, :],
                        rhs=ind4.rearrange("k a b -> k (a b)"),
                        start=True,
                        stop=False,
                    )
                    for mt in range(4):  # Wx_rz
                        for kx in range(2):
                            nc.tensor.matmul(
                                out=rz[:, mt, :],
                                lhsT=wxt_sb[:, kx * 6 + mt, :],
                                rhs=xt_sb[:, kx, j, g, :],
                                start=False,
                                # at r=0 the Wh_rz matmuls are skipped, so
                                # the rz accumulation group ends here
                                stop=(r == 0 and mt == 3 and kx == 1),
                            )
                for g in range(NG):
                    # Wx_n after all rz work: their WAR on the previous
                    # round's rn/an readers gets maximal slack
                    nn = nn_all[g]
                    for mt in range(4, 6):  # Wx_n -> nn regions 2:4
                        for kx in range(2):
                            nc.tensor.matmul(
                                out=nn[:, mt - 2, :],
                                lhsT=wxt_sb[:, kx * 6 + mt, :],
                                rhs=xt_sb[:, kx, j, g, :],
                                start=(kx == 0),
                                stop=False,
                            )
                for g in range(NG):
                    # --- PE recurrent phase: Wh (rz accum, hn fresh) ---
                    # At r=0 the state is exactly zero: skip the rz-side Wh
                    # matmuls (they add nothing); keep Wh_n for the region
                    # start=True init.
                    rz, nn = rz_all[g], nn_all[g]
                    hp = h_prev[g]
                    for mt in range(6):
                        if r == 0 and mt < 4:
                            continue
                        for kc in range(2):
                            nc.tensor.matmul(
                                out=rz[:, mt, :] if mt < 4 else nn[:, mt - 4, :],
                                lhsT=wht_sb[:, kc * 6 + mt, :],
                                rhs=hp[:, kc, :],
                                start=(mt >= 4 and kc == 0),
                                stop=(mt == 3 and kc == 1)
                                if mt < 4
                                else (mt == 5 and kc == 1),
                            )

                sgl = [None] * NG
                rnl = [None] * NG
                anl = [None] * NG
                ntl = [None] * NG
                dl = [None] * NG

                def em_sig(g):
                    # sg = [r0, r1, z'0, z'1]
                    sg = ew.tile([128, 4, LAN], F16, tag=f"sg{g}")
                    nc.scalar.activation(
                        out=sg, in_=rz_all[g], func=AF.Sigmoid
                    )
                    sgl[g] = sg

                def em_rn(g):
                    # rn = (ps_hn + bhn) * r   (2 stt, per-partition bias)
                    rn = ew.tile([128, 2, LAN], F16, tag=f"rn{g}")
                    for kc in range(2):
                        nc.vector.scalar_tensor_tensor(
                            out=rn[:, kc, :],
                            in0=nn_all[g][:, kc, :],
                            scalar=bhn2[:, kc : kc + 1],
                            in1=sgl[g][:, kc, :],
                            op0=AL.add,
                            op1=AL.mult,
                        )
                    rnl[g] = rn

                def em_an(g):
                    # an = (ps_xn + bxn) + rn  (2 stt)
                    an = ew.tile([128, 2, LAN], F16, tag=f"an{g}")
                    for kc in range(2):
                        nc.vector.scalar_tensor_tensor(
                            out=an[:, kc, :],
                            in0=nn_all[g][:, 2 + kc, :],
                            scalar=bxn2[:, kc : kc + 1],
                            in1=rnl[g][:, kc, :],
                            op0=AL.add,
                            op1=AL.add,
                        )
                    anl[g] = an

                def em_tanh(g):
                    nt = ew.tile([128, 2, LAN], F16, tag=f"nt{g}")
                    nc.scalar.activation(out=nt, in_=anl[g], func=AF.Tanh)
                    ntl[g] = nt

                def em_d(g):
                    # d = n - h ; e = d * z'  (DVE, sbuf 2x, back-to-back)
                    d = ew.tile([128, 2, LAN], F16, tag=f"d{g}")
                    nc.vector.tensor_tensor(
                        out=d, in0=ntl[g], in1=h_prev[g], op=AL.subtract
                    )
                    e = ew.tile([128, 2, LAN], F16, tag=f"e{g}")
                    nc.vector.tensor_tensor(
                        out=e, in0=d, in1=sgl[g][:, 2:4, :], op=AL.mult
                    )
                    dl[g] = e

                def em_tail(g):
                    # h' = h + e.  g0 on POOL (its deadline is earliest and
                    # POOL latency fits); g1-g3 on DVE right after their e so
                    # late-round Wh matmuls are not gated by the POOL hop.
                    dst = outc[:, j, :, g, :]
                    eng = nc.vector if g >= 1 else nc.gpsimd
                    eng.tensor_tensor(
                        out=dst, in0=h_prev[g], in1=dl[g], op=AL.add,
                    )
                    h_prev[g] = dst

                # software-pipelined interleave across the 4 chains; DVE
                # order completes ALL rn/an (they gate next round's PE
                # writes) before the d/e tail ops
                em_sig(0)
                em_rn(0)
                em_an(0)
                em_sig(1)
                em_tanh(0)
                em_rn(1)
                em_an(1)
                em_d(0)
                em_sig(2)
                em_tail(0)
                em_tanh(1)
                em_rn(2)
                em_an(2)
                em_d(1)
                em_sig(3)
                em_tail(1)
                em_tanh(2)
                em_rn(3)
                em_an(3)
                em_d(2)
                em_tail(2)
                em_tanh(3)
                em_d(3)
                em_tail(3)

                if r == W - 1:
                    # zero post-warmup state of boundary streams
                    for g in range(NG):
                        hm = outc[:, j, :, g, :]
                        nc.vector.tensor_tensor(
                            out=hm, in0=hm, in1=hmask_sb[:, :, g, :],
                            op=AL.mult,
                        )
                        h_prev[g] = hm
            nc.sync.dma_start(out=out[ch], in_=outc)
            del outc_prev

    _fix_drain_waits(nc)
    return nc


_CACHE = {}


def _get_nc(T=T_FULL):
    assert T == T_FULL, "v2 kernel is specialized to T=2048"
    if T not in _CACHE:
        _CACHE[T] = _build()
    return _CACHE[T]


def prep_in_maps(x, Wx, Wh, bx, bh):
    x = np.asarray(x, np.float32)
    Wx = np.asarray(Wx, np.float32).copy()
    Wh = np.asarray(Wh, np.float32).copy()
    bx = np.asarray(bx, np.float32)
    bh = np.asarray(bh, np.float32)
    T = x.shape[1]
    assert T == T_FULL

    # negate z-gate rows so sigmoid gives z' = 1-z
    Wx[:, 256:512, :] *= -1.0
    Wh[:, 256:512, :] *= -1.0
    brz = (bx + bh)[:, 0:512].copy()
    brz[:, 256:512] *= -1.0

    rr = np.arange(R)
    in_maps = []
    for c in range(8):
        l, k = c // 4, c % 4
        # chains: g0: fwd w 8k+0..3; g1: fwd w 8k+4..7; g2/g3: bwd same
        tidx = np.empty((NG, NWIN, R), np.int64)
        for g in range(NG):
            fwd = g < 2
            for ws in range(NWIN):
                w = 8 * k + (g % 2) * 4 + ws
                if fwd:
                    t = SEG * w - W + rr
                else:
                    t = SEG * (w + 1) - 1 + W - rr
                tidx[g, ws] = np.clip(t, 0, T - 1)
        # gather: [b, g, ws, r, d] -> [ch, p, kx, j, g, (ws, b)]
        xg = x[:, tidx, :].astype(np.float16)  # (32, NG, NWIN, R, 256)
        xg = xg.reshape(BS, NG, NWIN, NCH, C, 2, 128)
        xt_h = np.ascontiguousarray(xg.transpose(3, 6, 5, 4, 1, 2, 0)).reshape(
            NCH, 128, 2, C, NG, LAN
        )

        wht_h = np.ascontiguousarray(
            Wh[l].reshape(6, 128, 2, 128).transpose(3, 2, 0, 1).reshape(128, 12, 128),
            np.float16,
        )
        wxt_h = np.ascontiguousarray(
            Wx[l].reshape(6, 128, 2, 128).transpose(3, 2, 0, 1).reshape(128, 12, 128),
            np.float16,
        )
        b4 = brz[l].reshape(4, 128).astype(np.float16)
        ind4_h = np.zeros((4, 4, LAN), np.float16)
        for kk in range(4):
            ind4_h[kk, kk, :] = 1.0
        bhn2_h = np.ascontiguousarray(bh[l, 512:768].reshape(2, 128).T, np.float32)
        bxn2_h = np.ascontiguousarray(bx[l, 512:768].reshape(2, 128).T, np.float32)

        hm = np.ones((128, 2, NG, LAN), np.float16)
        if k == 0:
            hm[:, :, 0, 0:32] = 0.0  # fwd window 0
        if k == 3:
            hm[:, :, 3, 96:128] = 0.0  # bwd window 31
        in_maps.append(
            {
                "xt": xt_h,
                "wht": wht_h,
                "wxt": wxt_h,
                "b4d": b4,
                "ind4d": ind4_h,
                "bhn2d": bhn2_h,
                "bxn2d": bxn2_h,
                "hmask": hm,
            }
        )
    return in_maps


def assemble_out(per_core_out, T=T_FULL):
    OUT = np.empty((BS, T * L, 2 * H), np.float32)
    for c in range(8):
        l, k = c // 4, c % 4
        o = np.asarray(per_core_out[c], np.float32).reshape(NCH, 128, C, 2, NG, LAN)
        # [ch, p, j, kc, g, lane] -> [r, kc, p, g, ws, b]
        o = o.transpose(0, 2, 3, 1, 4, 5).reshape(R, 2, 128, NG, NWIN, BS)
        o = o.reshape(R, 256, NG, NWIN, BS)
        kept = o[W : W + SEG]  # [seg_j, 256, NG, NWIN, b]
        for g in range(NG):
            fwd = g < 2
            for ws in range(NWIN):
                w = 8 * k + (g % 2) * 4 + ws
                hs = kept[:, :, g, ws, :]  # [seg_j, 256, b]
                if not fwd:
                    hs = hs[::-1]
                ts = np.arange(SEG * w, SEG * (w + 1))
                col0 = 0 if fwd else 256
                OUT[:, 2 * ts + l, col0 : col0 + 256] = hs.transpose(2, 0, 1)
    return OUT


def kernel(x, Wx, Wh, bx, bh):
    T = x.shape[1]
    nc = _get_nc(T)
    in_maps = prep_in_maps(x, Wx, Wh, bx, bh)
    res = run_bass_kernel_spmd(nc, in_maps, core_ids=list(range(8)))
    kernel.last_results = res
    return assemble_out([r["out"] for r in res.results], T)


# revision 10
# speedup vs baseline: 1.0313x; 1.0214x over previous
"""Bidirectional 2-layer GRU (BS=32, T=2048, D=H=256) on 8 trn2 NeuronCores.

Time-parallel recurrence with warmup (GRU z-gate contraction makes a segment
started from h=0 converge to the true trajectory; W=12 -> ~3e-3 err).

v2 layout: P=32 windows of SEG=64 steps (+W warmup -> R=76 rounds), core
c = (layer c//4, k = c%4); chain g of core k owns 4 windows x 32 batch =
LAN=128 lanes (g0/g1 fwd w 8k..8k+7, g2/g3 bwd same; bwd streams are
host-pre-reversed). NG=4 chains hide the per-step dependency latency while
LAN=128 tiles keep fixed per-instruction overheads small. Engine balance
(per chain-round, steady-state cost-model):
  PE   ~1493ns: K=4-indicator rz-bias seed (fp16, free=512) + 8 Wx_rz +
         4 Wx_n + 12 Wh matmuls, f128/K=128. z-gate rows of Wx/Wh and bz
         are negated on host so sigmoid directly yields z' = 1-z.
  ACT  ~1060ns: sigmoid(ps_rz)->[r,z'], tanh(an)->n.
  DVE  ~1460ns: rn=(ps_hn+bhn)*r and an=(ps_xn+bxn)+rn as 2x
         scalar_tensor_tensor with per-partition bias columns; d=n-h and
         e=d*z' (fp16 sbuf 2x); h' = h+e for g1-g3 (right after e, so the
         round-tail Wh matmuls are not gated by a Pool hop).
  POOL : h' for g0 only (TensorScalarPtr is not legal on Pool; plain tt).
PSUM: per chain rz[128,4,128] + nn[128,4,128] fp32 = 2 banks -> all 8 banks,
bufs=1. Separate rz/nn tiles keep cross-round WAR waits fine-grained (a
fused tile serialized next-round Wx on the slowest previous-round reader).
PE emission: [seed+Wx_rz]*4, [Wx_n]*4 (max WAR slack), [Wh]*4; Wh_rz are
skipped at r=0 (h=0). Boundary streams (fwd w0 / bwd w31) get post-warmup
state zeroed by a mask multiply at round W-1. x is DMA'd per C=4-round
chunk in consumption order; outputs stream out per chunk.
"""

import os
from contextlib import ExitStack

import numpy as np

import concourse.bass as bass
from concourse import mybir
from concourse.alu_op_type import AluOpType
from concourse.tile import TileContext
from concourse.bass_utils import run_bass_kernel_spmd

BS, T_FULL, D = 32, 2048, 256
H, L = 256, 2
NG = 4            # chains per core
LAN = 128         # lanes per chain (4 windows x 32 batch)
NWIN = 4          # windows per chain
PW = 32           # windows per direction
SEG = T_FULL // PW  # 64 steps per window
W = 10            # warmup rounds
R = SEG + W       # 76 rounds
C = 2             # rounds per DMA chunk
NCH = R // C      # 19

F16 = mybir.dt.float16
F32 = mybir.dt.float32
AF = mybir.ActivationFunctionType
AL = AluOpType


def _fix_drain_waits(nc, max_waits=1):
    """Walrus rejects instructions with >1 sync-wait: split extras onto
    single-wait NOPs just before, on the same engine."""
    n_new = 0
    for f in nc.m.functions:
        for bb in f.blocks:
            insts = list(bb.instructions)
            out = []
            changed = False
            for inst in insts:
                si = inst.sync_info
                if si and len(si.on_wait) > max_waits:
                    waits = list(si.on_wait)
                    for k, w in enumerate(waits[:-max_waits]):
                        nd = mybir.InstNoOp(name=f"{inst.name}-w{k}", ins=[], outs=[])
                        nd.engine = inst.engine
                        nd.sync_info = mybir.SyncInfo(on_wait=[w], on_update=[])
                        out.append(nd)
                        nc.register_instruction(nd, overwrite=True)
                        n_new += 1
                    inst.sync_info = mybir.SyncInfo(
                        on_wait=waits[-max_waits:], on_update=list(si.on_update)
                    )
                    changed = True
                out.append(inst)
            if changed:
                lst = bb.instructions
                lst.clear()
                lst.extend(out)
                assert [i.name for i in bb.instructions] == [i.name for i in out]
    return n_new


def _build():
    nc = bass.Bass(name="bidir_gru_v2", trn_type="TRN2")

    # x in consumption order: [ch, p, kx, j, g, lane]
    xt = nc.dram_tensor("xt", [NCH, 128, 2, C, NG, LAN], F16, kind="ExternalInput")
    # weights [p(K-half), kc*6+mt, gate-col] with z-rows negated
    wht = nc.dram_tensor("wht", [128, 12, 128], F16, kind="ExternalInput")
    wxt = nc.dram_tensor("wxt", [128, 12, 128], F16, kind="ExternalInput")
    # K=4 rz-bias seed: b4[reg, p] = bias value of gate-tile reg, col p
    b4d = nc.dram_tensor("b4d", [4, 128], F16, kind="ExternalInput")
    ind4d = nc.dram_tensor("ind4d", [4, 4, LAN], F16, kind="ExternalInput")
    # n-side biases as per-partition scalar columns
    bhn2d = nc.dram_tensor("bhn2d", [128, 2], F32, kind="ExternalInput")
    bxn2d = nc.dram_tensor("bxn2d", [128, 2], F32, kind="ExternalInput")
    hmask = nc.dram_tensor("hmask", [128, 2, NG, LAN], F16, kind="ExternalInput")
    # out[ch, p, j, kc, g, lane]
    out = nc.dram_tensor("out", [NCH, 128, C, 2, NG, LAN], F16, kind="ExternalOutput")

    with TileContext(nc) as tc, ExitStack() as ctx:
        const = ctx.enter_context(tc.tile_pool(name="const", bufs=1))
        xtp = ctx.enter_context(tc.tile_pool(name="xtp", bufs=2))
        outp = ctx.enter_context(tc.tile_pool(name="outp", bufs=2))
        psp = ctx.enter_context(tc.tile_pool(name="psp", bufs=1, space="PSUM"))
        ew = ctx.enter_context(tc.tile_pool(name="ew", bufs=3))

        b4_sb = const.tile([4, 128], F16)
        nc.sync.dma_start(out=b4_sb, in_=b4d[:, :])
        ind4 = const.tile([4, 4, LAN], F16)
        nc.sync.dma_start(out=ind4, in_=ind4d[:, :, :])
        wxt_sb = const.tile([128, 12, 128], F16)
        nc.sync.dma_start(out=wxt_sb, in_=wxt[:, :, :])
        wht_sb = const.tile([128, 12, 128], F16)
        nc.sync.dma_start(out=wht_sb, in_=wht[:, :, :])
        bhn2 = const.tile([128, 2], F32)
        nc.sync.dma_start(out=bhn2, in_=bhn2d[:, :])
        bxn2 = const.tile([128, 2], F32)
        nc.sync.dma_start(out=bxn2, in_=bxn2d[:, :])
        hmask_sb = const.tile([128, 2, NG, LAN], F16)
        nc.sync.dma_start(out=hmask_sb, in_=hmask[:, :, :, :])
        zeros = const.tile([128, 2, NG, LAN], F16)
        nc.vector.memset(zeros, 0.0)

        h_prev = [zeros[:, :, g, :] for g in range(NG)]

        outc = None
        for ch in range(NCH):
            xt_sb = xtp.tile([128, 2, C, NG, LAN], F16, tag="xt")
            nc.sync.dma_start(out=xt_sb, in_=xt[ch])
            outc_prev = outc
            outc = outp.tile([128, C, 2, NG, LAN], F16, tag="outc")
            for j in range(C):
                r = ch * C + j
                rz_all = []
                nn_all = []
                for g in range(NG):
                    # --- PE h-independent phase: rz-bias seed + Wx_rz ---
                    # rz tile: regions 0:4 = [r0, r1, z'0, z'1] (1 PSUM bank)
                    # nn tile: regions 0:2 = hn, 2:4 = xn (1 PSUM bank)
                    rz = psp.tile([128, 4, LAN], F32, tag=f"rz{g}")
                    nn = psp.tile([128, 4, LAN], F32, tag=f"nn{g}")
                    rz_all.append(rz)
                    nn_all.append(nn)
                    # seed rz with biases (z rows negated)
                    nc.tensor.matmul(
                        out=rz.rearrange("p a b -> p (a b)"),
                        lhsT=b4_sb[:, :],
                        rhs=ind4.rearrange("k a b -> k (a b)"),
                        start=True,
                        stop=False,
                    )
                    for mt in range(4):  # Wx_rz
                        for kx in range(2):
                            nc.tensor.matmul(
                                out=rz[:, mt, :],
                                lhsT=wxt_sb[:, kx * 6 + mt, :],
                                rhs=xt_sb[:, kx, j, g, :],
                                start=False,
                                # at r=0 the Wh_rz matmuls are skipped, so
                                # the rz accumulation group ends here
                                stop=(r == 0 and mt == 3 and kx == 1),
                            )
                for g in range(NG):
                    # Wx_n after all rz work: their WAR on the previous
                    # round's rn/an readers gets maximal slack
                    nn = nn_all[g]
                    for mt in range(4, 6):  # Wx_n -> nn regions 2:4
                        for kx in range(2):
                            nc.tensor.matmul(
                                out=nn[:, mt - 2, :],
                                lhsT=wxt_sb[:, kx * 6 + mt, :],
                                rhs=xt_sb[:, kx, j, g, :],
                                start=(kx == 0),
                                stop=False,
                            )
                for g in range(NG):
                    # --- PE recurrent phase: Wh (rz accum, hn fresh) ---
                    # At r=0 the state is exactly zero: skip the rz-side Wh
                    # matmuls (they add nothing); keep Wh_n for the region
                    # start=True init.
                    rz, nn = rz_all[g], nn_all[g]
                    hp = h_prev[g]
                    for mt in range(6):
                        if r == 0 and mt < 4:
                            continue
                        for kc in range(2):
                            nc.tensor.matmul(
                                out=rz[:, mt, :] if mt < 4 else nn[:, mt - 4, :],
                                lhsT=wht_sb[:, kc * 6 + mt, :],
                                rhs=hp[:, kc, :],
                                start=(mt >= 4 and kc == 0),
                                stop=(mt == 3 and kc == 1)
                                if mt < 4
                                else (mt == 5 and kc == 1),
                            )

                sgl = [None] * NG
                rnl = [None] * NG
                anl = [None] * NG
                ntl = [None] * NG
                dl = [None] * NG

                def em_sig(g):
                    # sg = [r0, r1, z'0, z'1]
                    sg = ew.tile([128, 4, LAN], F16, tag=f"sg{g}")
                    nc.scalar.activation(
                        out=sg, in_=rz_all[g], func=AF.Sigmoid
                    )
                    sgl[g] = sg

                def em_rn(g):
                    # rn = (ps_hn + bhn) * r   (2 stt, per-partition bias)
                    rn = ew.tile([128, 2, LAN], F16, tag=f"rn{g}")
                    for kc in range(2):
                        nc.vector.scalar_tensor_tensor(
                            out=rn[:, kc, :],
                            in0=nn_all[g][:, kc, :],
                            scalar=bhn2[:, kc : kc + 1],
                            in1=sgl[g][:, kc, :],
                            op0=AL.add,
                            op1=AL.mult,
                        )
                    rnl[g] = rn

                def em_an(g):
                    # an = (ps_xn + bxn) + rn  (2 stt)
                    an = ew.tile([128, 2, LAN], F16, tag=f"an{g}")
                    for kc in range(2):
                        nc.vector.scalar_tensor_tensor(
                            out=an[:, kc, :],
                            in0=nn_all[g][:, 2 + kc, :],
                            scalar=bxn2[:, kc : kc + 1],
                            in1=rnl[g][:, kc, :],
                            op0=AL.add,
                            op1=AL.add,
                        )
                    anl[g] = an

                def em_tanh(g):
                    nt = ew.tile([128, 2, LAN], F16, tag=f"nt{g}")
                    nc.scalar.activation(out=nt, in_=anl[g], func=AF.Tanh)
                    ntl[g] = nt

                def em_d(g):
                    # d = n - h ; e = d * z'  (DVE, sbuf 2x, back-to-back)
                    d = ew.tile([128, 2, LAN], F16, tag=f"d{g}")
                    nc.vector.tensor_tensor(
                        out=d, in0=ntl[g], in1=h_prev[g], op=AL.subtract
                    )
                    e = ew.tile([128, 2, LAN], F16, tag=f"e{g}")
                    nc.vector.tensor_tensor(
                        out=e, in0=d, in1=sgl[g][:, 2:4, :], op=AL.mult
                    )
                    dl[g] = e

                def em_tail(g):
                    # h' = h + e.  g0 on POOL (its deadline is earliest and
                    # POOL latency fits); g1-g3 on DVE right after their e so
                    # late-round Wh matmuls are not gated by the POOL hop.
                    dst = outc[:, j, :, g, :]
                    eng = nc.vector if g >= 1 else nc.gpsimd
                    eng.tensor_tensor(
                        out=dst, in0=h_prev[g], in1=dl[g], op=AL.add,
                    )
                    h_prev[g] = dst

                # software-pipelined interleave across the 4 chains; DVE
                # order completes ALL rn/an (they gate next round's PE
                # writes) before the d/e tail ops
                em_sig(0)
                em_rn(0)
                em_an(0)
                em_sig(1)
                em_tanh(0)
                em_rn(1)
                em_an(1)
                em_d(0)
                em_sig(2)
                em_tail(0)
                em_tanh(1)
                em_rn(2)
                em_an(2)
                em_d(1)
                em_sig(3)
                em_tail(1)
                em_tanh(2)
                em_rn(3)
                em_an(3)
                em_d(2)
                em_tail(2)
                em_tanh(3)
                em_d(3)
                em_tail(3)

                if r == W - 1:
                    # zero post-warmup state of boundary streams
                    for g in range(NG):
                        hm = outc[:, j, :, g, :]
                        nc.vector.tensor_tensor(
                            out=hm, in0=hm, in1=hmask_sb[:, :, g, :],
                            op=AL.mult,
                        )
                        h_prev[g] = hm
            nc.sync.dma_start(out=out[ch], in_=outc)
            del outc_prev

    _fix_drain_waits(nc)
    return nc


_CACHE = {}


def _get_nc(T=T_FULL):
    assert T == T_FULL, "v2 kernel is specialized to T=2048"
    if T not in _CACHE:
        _CACHE[T] = _build()
    return _CACHE[T]


def prep_in_maps(x, Wx, Wh, bx, bh):
    x = np.asarray(x, np.float32)
    Wx = np.asarray(Wx, np.float32).copy()
    Wh = np.asarray(Wh, np.float32).copy()
    bx = np.asarray(bx, np.float32)
    bh = np.asarray(bh, np.float32)
    T = x.shape[1]
    assert T == T_FULL

    # negate z-gate rows so sigmoid gives z' = 1-z
    Wx[:, 256:512, :] *= -1.0
    Wh[:, 256:512, :] *= -1.0
    brz = (bx + bh)[:, 0:512].copy()
    brz[:, 256:512] *= -1.0

    rr = np.arange(R)
    in_maps = []
    for c in range(8):
        l, k = c // 4, c % 4
        # chains: g0: fwd w 8k+0..3; g1: fwd w 8k+4..7; g2/g3: bwd same
        tidx = np.empty((NG, NWIN, R), np.int64)
        for g in range(NG):
            fwd = g < 2
            for ws in range(NWIN):
                w = 8 * k + (g % 2) * 4 + ws
                if fwd:
                    t = SEG * w - W + rr
                else:
                    t = SEG * (w + 1) - 1 + W - rr
                tidx[g, ws] = np.clip(t, 0, T - 1)
        # gather: [b, g, ws, r, d] -> [ch, p, kx, j, g, (ws, b)]
        xg = x[:, tidx, :].astype(np.float16)  # (32, NG, NWIN, R, 256)
        xg = xg.reshape(BS, NG, NWIN, NCH, C, 2, 128)
        xt_h = np.ascontiguousarray(xg.transpose(3, 6, 5, 4, 1, 2, 0)).reshape(
            NCH, 128, 2, C, NG, LAN
        )

        wht_h = np.ascontiguousarray(
            Wh[l].reshape(6, 128, 2, 128).transpose(3, 2, 0, 1).reshape(128, 12, 128),
            np.float16,
        )
        wxt_h = np.ascontiguousarray(
            Wx[l].reshape(6, 128, 2, 128).transpose(3, 2, 0, 1).reshape(128, 12, 128),
            np.float16,
        )
        b4 = brz[l].reshape(4, 128).astype(np.float16)
        ind4_h = np.zeros((4, 4, LAN), np.float16)
        for kk in range(4):
            ind4_h[kk, kk, :] = 1.0
        bhn2_h = np.ascontiguousarray(bh[l, 512:768].reshape(2, 128).T, np.float32)
        bxn2_h = np.ascontiguousarray(bx[l, 512:768].reshape(2, 128).T, np.float32)

        hm = np.ones((128, 2, NG, LAN), np.float16)
        if k == 0:
            hm[:, :, 0, 0:32] = 0.0  # fwd window 0
        if k == 3:
            hm[:, :, 3, 96:128] = 0.0  # bwd window 31
        in_maps.append(
            {
                "xt": xt_h,
                "wht": wht_h,
                "wxt": wxt_h,
                "b4d": b4,
                "ind4d": ind4_h,
                "bhn2d": bhn2_h,
                "bxn2d": bxn2_h,
                "hmask": hm,
            }
        )
    return in_maps


def assemble_out(per_core_out, T=T_FULL):
    OUT = np.empty((BS, T * L, 2 * H), np.float32)
    for c in range(8):
        l, k = c // 4, c % 4
        o = np.asarray(per_core_out[c], np.float32).reshape(NCH, 128, C, 2, NG, LAN)
        # [ch, p, j, kc, g, lane] -> [r, kc, p, g, ws, b]
        o = o.transpose(0, 2, 3, 1, 4, 5).reshape(R, 2, 128, NG, NWIN, BS)
        o = o.reshape(R, 256, NG, NWIN, BS)
        kept = o[W : W + SEG]  # [seg_j, 256, NG, NWIN, b]
        for g in range(NG):
            fwd = g < 2
            for ws in range(NWIN):
                w = 8 * k + (g % 2) * 4 + ws
                hs = kept[:, :, g, ws, :]  # [seg_j, 256, b]
                if not fwd:
                    hs = hs[::-1]
                ts = np.arange(SEG * w, SEG * (w + 1))
                col0 = 0 if fwd else 256
                OUT[:, 2 * ts + l, col0 : col0 + 256] = hs.transpose(2, 0, 1)
    return OUT


def kernel(x, Wx, Wh, bx, bh):
    T = x.shape[1]
    nc = _get_nc(T)
    in_maps = prep_in_maps(x, Wx, Wh, bx, bh)
    res = run_bass_kernel_spmd(nc, in_maps, core_ids=list(range(8)))
    kernel.last_results = res
    return assemble_out([r["out"] for r in res.results], T)
